# revision 5
# baseline (speedup 1.0000x reference)
"""Multi-head self-attention Trainium2 kernel (Bass/Tile), v7.

Problem: x:(8,256,32,32), 8 heads, head_dim=32, N=H*W=1024.
Sharding: data-parallel over batch B=8 -> one batch element per NeuronCore.

Design vs v6 (135.8us local):
  - 32-pitch head layout: Qh/Kh [128,1024] x 2 tiles, head 4t+j at
    partitions 32j.  Halves Q/K projection streaming (no zero columns).
  - q-bias via DVE tensor_scalar_add (bq folded, scaled); no u-row/ones-row.
    S = (Wq x * s + bq s)^T (Wk x): bk/bq constants drop under softmax.
  - Steady state is ACT-paced: per (pair, mc): 2 score-pair MMs (K=32,
    2-way row-group concurrent), 2 exps [128,1024], 1 AV pair (M=33,
    2-way col-group concurrent, N=1024 streams).  PE/slot ~= 1.0us cold
    < 1.11us exp, so ACT never waits on PE even at K=4/8.
  - Garbage-matmul warm-up at t=0 (scratch SBUF) warms HAM during input DMA
    (an exp-paced stream never warms it; steady state runs at 1.2GHz).
  - Projections: K0/Q0 half-granular on the ps rotation feed the first
    score; V and K1/Q1 injected into pair-0 slot shadows.
  - PSUM: ps 2x[128,1024] (score double-buffer, 4 banks) + psO 2x[98,512]
    (per-jn AV accum so next pair's AV never waits the drain; esum rows
    ride at partitions 32/96 via VH ones-cols) + aux [128,*] (proj/prn/po
    transients, 2 banks) = 8 banks.  MM outputs are capped at 512 f32
    cols (one-bank ISA rule); concurrent same-bank MM writes wedge the
    device.
  - Per-pair epilogue: DVE copies psO -> OST (partition-aligned, no
    remap), DMA-gathers esum rows to base-0 ESUM (recip_approx_fast
    miscomputes on base!=0 APs), recip+bf16 cast, per-pair broadcast MM
    (prn), O1_p = OST*prn, pair-split output projection (WOP_p has zero
    rows at esum/junk positions so junk*0 stays clean), PART[mo]
    accumulated on DVE.  Pair 3 runs this chain jn-pipelined, copies
    split ACT/DVE, with quarter-wise output DMA on separate queues.
"""

import math

import numpy as np
import ml_dtypes

import concourse.bass as bass
import concourse.mybir as mybir
import concourse.tile as tile
from concourse import bacc
from concourse.bass_utils import run_bass_kernel_spmd

F32 = mybir.dt.float32
BF16 = mybir.dt.bfloat16
EXP = mybir.ActivationFunctionType.Exp

NH = 8
HD = 32
C = 256
N = 1024
NCORES = 8

_NC = None
LAST_RESULTS = None
import os as _os
KEEPALIVE = int(_os.environ.get("BASS_KEEPALIVE", "0"))


def _emit(tc, io):
    nc = tc.nc
    import contextlib

    ctx = contextlib.ExitStack()
    with ctx:
        pers = ctx.enter_context(tc.tile_pool(name="pers", bufs=1))
        etp = ctx.enter_context(tc.tile_pool(name="etp", bufs=3))
        psp = ctx.enter_context(tc.tile_pool(name="psp", bufs=1, space="PSUM"))

        def ptile(name, shape, dtype=F32):
            return pers.tile(shape, dtype, tag=name, name=name)

        # ---------------- persistent tiles ----------------
        XB = [ptile(f"XB{i}", [128, N], BF16) for i in range(2)]
        WQ = [ptile(f"WQ{i}", [128, C], BF16) for i in range(2)]
        WK = [ptile(f"WK{i}", [128, C], BF16) for i in range(2)]
        WV = [ptile(f"WV{i}", [128, C], BF16) for i in range(2)]
        WOP = [ptile(f"WOP{p}", [97, C], BF16) for p in range(4)]
        BQ = ptile("BQ", [128, 2])
        OHB = ptile("OHB", [2, 97], BF16)
        XPB = [ptile(f"XPB{i}", [128, N]) for i in range(2)]
        Qh = [ptile(f"Qh{t}", [128, N], BF16) for t in range(2)]
        Kh = [ptile(f"Kh{t}", [128, N], BF16) for t in range(2)]
        VH = [ptile(f"VH{mc}", [128, NH * 33], BF16) for mc in range(NH)]
        OST = [ptile(f"OST{p}", [97, N]) for p in range(4)]
        ESUM = [ptile(f"ESUM{p}", [2, N]) for p in range(4)]
        RECB = [ptile(f"RECB{p}", [2, N], BF16) for p in range(4)]
        O1 = [ptile(f"O1{p}", [97, N], BF16) for p in range(4)]
        PART = [ptile(f"PART{mo}", [128, N]) for mo in range(2)]
        SCR = ptile("SCR", [128, 640], BF16)

        # ---------------- warm-up + input DMA ----------------
        # garbage matmuls on scratch SBUF warm the PE HAM clock-gate to
        # 8/8 while the first-wave DMAs land (nothing depends on them)
        nc.vector.memset(SCR[:], 0.25)
        # OST rows 33-63 are never written by the psO drains; zero them once
        # so the O1 multiply reads initialized data (prn rows are 0 there)
        for p in range(4):
            nc.vector.memset(OST[p][32:64, :], 0.0)
        wup = psp.tile([128, N], F32, tag="aux", bufs=1, name="wup")
        for i in range(12):
            nc.tensor.matmul(
                wup[:, 0:512] if i % 2 == 0 else wup[:, 512:1024],
                SCR[:, 0:128], SCR[:, 128:640],
                start=True, stop=True,
            )
        nc.vector.tensor_copy(SCR[0:1, 0:32], wup[0:1, 0:32])

        # first-wave loads, issue spread over engine sequencers
        nc.sync.dma_start(XB[0][:], io["xb"][0:128, :])
        nc.sync.dma_start(XB[1][:], io["xb"][128:256, :])
        for i in range(2):
            nc.scalar.dma_start(WK[i][:], io["wk"][i * 128 : (i + 1) * 128, :])
        for i in range(2):
            nc.gpsimd.dma_start(WQ[i][:], io["wq"][i * 128 : (i + 1) * 128, :])
        nc.gpsimd.dma_start(BQ[:], io["bqc"])
        for i in range(2):
            nc.scalar.dma_start(WV[i][:], io["wv"][i * 128 : (i + 1) * 128, :])
        nc.gpsimd.dma_start(OHB[:], io["ohb"])

        # warm the ACT exp table while weights land
        warm = ptile("warm", [1, 32])
        nc.scalar.activation(warm[:], WK[0][0:1, 0:32], EXP)

        def late_dma():
            for p in range(4):
                nc.sync.dma_start(WOP[p][:], io["wop"][97 * p : 97 * p + 97, :])
            for i in range(2):
                nc.sync.dma_start(XPB[i][:], io["xpb"][i * 128 : (i + 1) * 128, :])

        # ---------------- building blocks ----------------
        def qk_half(t, jn, w, dst, bias, tag="aux"):
            """Half (512 pixels) of one 128-row chunk of the Q/K projection.
            Bias added on the psum->SBUF copy for Q."""
            js = slice(jn * 512, (jn + 1) * 512)
            pp = psp.tile(
                [128, 512], F32, tag=tag, bufs=1 if tag == "aux" else 2,
                name=f"pp{dst is Qh}{t}_{jn}",
            )
            for kc in range(2):
                nc.tensor.matmul(
                    pp[:], w[kc][:, t * 128 : (t + 1) * 128], XB[kc][:, js],
                    start=(kc == 0), stop=(kc == 1),
                )
            if bias is not None:
                nc.vector.tensor_scalar_add(dst[t][:, js], pp[:], bias[:, t : t + 1])
            else:
                nc.vector.tensor_copy(dst[t][:, js], pp[:])

        def pv_pack(i):
            """V^T for m-chunks 2i, 2i+1 in one aux psum alloc."""
            pvp = psp.tile([128, 512], F32, tag="aux", bufs=1, name=f"pvp{i}")
            for k in range(2):
                mc = 2 * i + k
                for kc in range(2):
                    nc.tensor.matmul(
                        pvp[:, k * 256 : (k + 1) * 256],
                        XB[kc][:, mc * 128 : (mc + 1) * 128],
                        WV[kc][:],
                        start=(kc == 0), stop=(kc == 1),
                    )
            for k in range(2):
                mc = 2 * i + k
                vh3 = VH[mc].rearrange("p (h c) -> p h c", c=33)
                pv3 = pvp[:, k * 256 : (k + 1) * 256].rearrange(
                    "p (h d) -> p h d", d=32
                )
                nc.vector.memset(VH[mc][:], 1.0)
                nc.vector.tensor_copy(vh3[:, :, 0:32], pv3[:, :, :])

        def score_mms(p, mc, jn):
            t, half = p // 2, p % 2
            ps = psp.tile([128, N], F32, tag="ps", bufs=2, name=f"ps{p}_{mc}_{jn}")
            for hh in range(2):
                base = 64 * half + 32 * hh
                nc.tensor.matmul(
                    ps[:, hh * 512 : (hh + 1) * 512],
                    Kh[t][base : base + 32, mc * 128 : (mc + 1) * 128],
                    Qh[t][base : base + 32, jn * 512 : (jn + 1) * 512],
                    start=True, stop=True,
                    tile_position=(base, 0),
                    skip_group_check=True,
                )
            return ps

        def exp_op(p, mc, jn, ps, et):
            nc.scalar.activation(et[:], ps[:], EXP)

        def av_mms(p, mc, jn, et, psO):
            for hh in range(2):
                h = 2 * p + hh
                nc.tensor.matmul(
                    psO[jn][64 * hh : 64 * hh + 33, :],
                    VH[mc][:, 33 * h : 33 * h + 33],
                    et[:, hh * 512 : (hh + 1) * 512],
                    start=(mc == 0), stop=(mc == 7),
                    tile_position=(0, 64 * hh),
                    skip_group_check=True,
                )

        def drain_jn(p, jn, psO, split_engines=False):
            """psO[jn] -> OST columns (partition-aligned) + esum DMA gather.
            Tiny esum-row copies go first so the gather DMAs fire early;
            for the tail-critical last drain the copies split across ACT
            (idle by then) and DVE."""
            js = slice(jn * 512, (jn + 1) * 512)
            copyA = nc.scalar.copy if split_engines else nc.vector.tensor_copy
            copyA(OST[p][32:33, js], psO[jn][32:33, :])
            nc.vector.tensor_copy(OST[p][96:97, js], psO[jn][96:97, :])
            nc.sync.dma_start(ESUM[p][0:1, js], OST[p][32:33, js], single_packet=True)
            nc.scalar.dma_start(ESUM[p][1:2, js], OST[p][96:97, js], single_packet=True)
            copyA(OST[p][0:32, js], psO[jn][0:32, :])
            nc.vector.tensor_copy(OST[p][64:96, js], psO[jn][64:96, :])

        def recip_jn(p, jn, cast_on_act=False):
            js = slice(jn * 512, (jn + 1) * 512)
            RECF = etp.tile([2, 512], F32, tag="recf", bufs=2, name=f"recf{p}{jn}")
            with nc.allow_low_precision("softmax denom recip (~1e-3 rel)"):
                nc.vector.reciprocal_approx_fast(RECF[:], ESUM[p][:, js])
            if cast_on_act:
                nc.scalar.copy(RECB[p][:, js], RECF[:])
            else:
                nc.vector.tensor_copy(RECB[p][:, js], RECF[:])

        def recip_pair(p):
            for jn in range(2):
                recip_jn(p, jn)

        prns = {}

        def prn_mms(p, jn=None, tag="aux"):
            """prn_p = OHB^T @ RECB[p] broadcast into 32-row blocks."""
            if p not in prns:
                prns[p] = psp.tile(
                    [128, N], F32, tag=tag, bufs=2 if tag == "ps" else 1,
                    name=f"prn{p}",
                )
            prn = prns[p]
            for j in ((0, 1) if jn is None else (jn,)):
                js = slice(j * 512, (j + 1) * 512)
                nc.tensor.matmul(
                    prn[0:97, js], OHB[:], RECB[p][:, js],
                    start=True, stop=True,
                    skip_group_check=True,
                )

        def norm_mul(p, jn=None):
            prn = prns[p]
            if jn is None:
                nc.vector.tensor_mul(O1[p][:], OST[p][:], prn[0:97, :])
            else:
                js = slice(jn * 512, (jn + 1) * 512)
                nc.vector.tensor_mul(O1[p][:, js], OST[p][:, js], prn[0:97, js])

        OUT_QUEUES = [nc.sync, nc.scalar, nc.gpsimd, nc.sync]

        po3 = {}

        def po_unit(p, mo, tag="aux", jn=None):
            """Pair p's contribution to output chunk mo; PART accumulates
            on DVE (PART[mo] starts as XPB[mo] + pair0).  For the final
            pair (jn-split) the adds/DMAs go out quarter-wise on separate
            queues."""
            if p == 3:
                if mo not in po3:
                    po3[mo] = psp.tile(
                        [128, N], F32, tag=tag, bufs=2, name=f"po3_{mo}"
                    )
                po = po3[mo]
                js = slice(jn * 512, (jn + 1) * 512)
                nc.tensor.matmul(
                    po[:, js], WOP[p][:, mo * 128 : (mo + 1) * 128],
                    O1[p][:, js],
                    start=True, stop=True,
                )
                nc.vector.tensor_add(PART[mo][:, js], po[:, js], PART[mo][:, js])
                OUT_QUEUES[2 * mo + jn].dma_start(
                    io["out"][mo * 128 : (mo + 1) * 128, js],
                    PART[mo][:, js],
                )
                return
            raise AssertionError("mid-stream pairs use po_half")

        pos = {}

        def po_half(p, mo, jn):
            """One jn-half of pair p's output-projection contribution."""
            if (p, mo) not in pos:
                pos[(p, mo)] = psp.tile(
                    [128, N], F32, tag="aux", bufs=1, name=f"po{p}_{mo}"
                )
            po = pos[(p, mo)]
            js = slice(jn * 512, (jn + 1) * 512)
            nc.tensor.matmul(
                po[:, js], WOP[p][:, mo * 128 : (mo + 1) * 128],
                O1[p][:, js],
                start=True, stop=True,
            )
            if p == 0:
                nc.vector.tensor_add(PART[mo][:, js], po[:, js], XPB[mo][:, js])
            else:
                nc.vector.tensor_add(PART[mo][:, js], po[:, js], PART[mo][:, js])

        # ---------------- emission schedule ----------------
        # first-score prerequisites, half-granular on the ps rotation
        qk_half(0, 0, WK, Kh, None, tag="ps")
        qk_half(0, 0, WQ, Qh, BQ, tag="ps")

        # deferred PE work injected into slot shadows
        deferred = {
            (0, 0): [lambda: pv_pack(0)],
            (0, 1): [lambda: pv_pack(1)],
            (0, 2): [lambda: pv_pack(2)],
            (0, 3): [lambda: pv_pack(3)],
            (0, 4): [lambda: qk_half(1, 0, WK, Kh, None)],
            (0, 5): [late_dma, lambda: qk_half(1, 0, WQ, Qh, BQ)],
            (0, 6): [lambda: qk_half(1, 1, WK, Kh, None)],
            (0, 7): [lambda: qk_half(1, 1, WQ, Qh, BQ)],
        }
        for q in range(3):
            deferred[(q + 1, 1)] = [lambda q=q: prn_mms(q, jn=0)]
            deferred[(q + 1, 2)] = [lambda q=q: (prn_mms(q, jn=1),
                                                 norm_mul(q))]
            deferred[(q + 1, 3)] = [lambda q=q: po_half(q, 0, 0)]
            deferred[(q + 1, 4)] = [lambda q=q: po_half(q, 0, 1)]
            deferred[(q + 1, 5)] = [lambda q=q: po_half(q, 1, 0)]
            deferred[(q + 1, 6)] = [lambda q=q: po_half(q, 1, 1)]

        tail3 = []
        for p in range(4):
            psO = [
                psp.tile([98, 512], F32, tag="psO", bufs=2, name=f"psO{p}_{jn}")
                for jn in range(2)
            ]
            for mc in range(8):
                ets = []
                for jn in range(2):
                    if (p, mc, jn) == (0, 0, 1):
                        # rest of chunk-0 projections right behind score 0
                        qk_half(0, 1, WK, Kh, None, tag="ps")
                        qk_half(0, 1, WQ, Qh, BQ, tag="ps")
                    ps = score_mms(p, mc, jn)
                    et = etp.tile([128, N], BF16, tag="et", name=f"et{p}_{mc}_{jn}")
                    exp_op(p, mc, jn, ps, et)
                    ets.append(et)
                for fn in deferred.get((p, mc), ()):
                    fn()
                for jn in range(2):
                    av_mms(p, mc, jn, ets[jn], psO)
                    if mc == 7 and p < 3:
                        drain_jn(p, jn, psO)
                    elif mc == 7:
                        # jn-pipelined tail, DVE/ACT balanced: esum copies +
                        # gather DMAs fire first; jn1's early copies run on
                        # the just-idle ACT; the recip DMA-wait is filled
                        # with block copies; casts go to ACT so DVE keeps
                        # only recip/mul/add on the critical chain.
                        js = slice(jn * 512, (jn + 1) * 512)
                        copyA = nc.scalar.copy if jn else nc.vector.tensor_copy
                        copyA(OST[3][32:33, js], psO[jn][32:33, :])
                        nc.vector.tensor_copy(OST[3][96:97, js], psO[jn][96:97, :])
                        nc.sync.dma_start(ESUM[3][0:1, js], OST[3][32:33, js], single_packet=True)
                        nc.scalar.dma_start(ESUM[3][1:2, js], OST[3][96:97, js], single_packet=True)
                        copyA(OST[3][0:32, js], psO[jn][0:32, :])
                        tail3.append(jn)

                def run_tail3():
                    # recips first (their DMA inputs land earliest), then
                    # blockB copies + both prn MMs ahead of any po unit so
                    # neither jn's normalize waits on the other's project
                    for jn in tail3:
                        recip_jn(3, jn, cast_on_act=True)
                    for jn in tail3:
                        js = slice(jn * 512, (jn + 1) * 512)
                        nc.vector.tensor_copy(OST[3][64:96, js], psO[jn][64:96, :])
                        prn_mms(3, jn=jn, tag="aux")
                    for jn in tail3:
                        norm_mul(3, jn)
                        po_unit(3, 0, tag="ps", jn=jn)
                        po_unit(3, 1, tag="ps", jn=jn)
                    tail3.clear()

                if p == 3 and mc == 7:
                    run_tail3()
                for _ in range(KEEPALIVE):
                    nc.tensor.ldweights(SCR[:, 0:128])
            if p < 3:
                recip_pair(p)


def build_nc():
    nc = bacc.Bacc("TRN2", target_bir_lowering=False, debug=False)
    io = {}
    for name, shape, dt_ in [
        ("xb", (C, N), BF16),
        ("wq", (C, C), BF16),
        ("wk", (C, C), BF16),
        ("wv", (C, C), BF16),
        ("wop", (388, C), BF16),
        ("bqc", (128, 2), F32),
        ("ohb", (2, 97), BF16),
        ("xpb", (C, N), F32),
    ]:
        io[name] = nc.dram_tensor(name, shape, dt_, kind="ExternalInput").ap()
    io["out"] = nc.dram_tensor("out", (C, N), F32, kind="ExternalOutput").ap()
    with tile.TileContext(nc) as tc:
        _emit(tc, io)
    nc.finalize()
    return nc


def host_prep(x, Wq, bq, Wk, bk, Wv, bv, Wo, bo):
    """Build per-core input maps (numpy only)."""
    bf16 = ml_dtypes.bfloat16
    x = np.ascontiguousarray(np.asarray(x, np.float32))
    Wq, bq = np.asarray(Wq, np.float32), np.asarray(bq, np.float32)
    Wk = np.asarray(Wk, np.float32)
    Wv, bv = np.asarray(Wv, np.float32), np.asarray(bv, np.float32)
    Wo, bo = np.asarray(Wo, np.float32), np.asarray(bo, np.float32)
    s = 1.0 / math.sqrt(HD)

    wq_hat = np.ascontiguousarray(Wq.T * s)          # [C, C] col 32h+d
    wk_hat = np.ascontiguousarray(Wk.T)
    bqc = (bq * s).reshape(2, 128).T                  # [128, 2] chunk cols
    wv_hat = np.ascontiguousarray(Wv.T)

    woT = Wo.T                                        # [C(d), C(out)]
    wop = np.zeros((388, C), np.float32)
    for p in range(4):
        wop[97 * p + 0 : 97 * p + 32, :] = woT[32 * (2 * p) : 32 * (2 * p) + 32, :]
        wop[97 * p + 64 : 97 * p + 96, :] = woT[32 * (2 * p + 1) : 32 * (2 * p + 1) + 32, :]

    ohb = np.zeros((2, 97), np.float32)
    ohb[0, 0:32] = 1.0
    ohb[1, 64:96] = 1.0

    bo2 = Wo @ bv + bo

    common = {
        "wq": wq_hat.astype(bf16),
        "wk": wk_hat.astype(bf16),
        "wv": wv_hat.astype(bf16),
        "wop": wop.astype(bf16),
        "bqc": np.ascontiguousarray(bqc),
        "ohb": ohb.astype(bf16),
    }

    B = x.shape[0]
    in_maps = []
    for b in range(B):
        xb = np.ascontiguousarray(x[b].reshape(C, N))
        m = dict(common)
        m["xb"] = xb.astype(bf16)
        m["xpb"] = np.ascontiguousarray(xb + bo2[:, None])
        in_maps.append(m)
    return in_maps


def kernel(x, Wq, bq, Wk, bk, Wv, bv, Wo, bo):
    global _NC, LAST_RESULTS
    if _NC is None:
        _NC = build_nc()
    in_maps = host_prep(x, Wq, bq, Wk, bk, Wv, bv, Wo, bo)
    res = run_bass_kernel_spmd(_NC, in_maps, core_ids=list(range(NCORES)))
    LAST_RESULTS = res
    out = np.stack([r["out"] for r in res.results], axis=0)
    return out.reshape(NCORES, C, 32, 32).astype(np.float32)


if __name__ == "__main__":
    rng = np.random.default_rng(0)
    ins = {
        "x": rng.standard_normal((8, C, 32, 32), dtype=np.float32),
        "Wq": rng.standard_normal((C, C), dtype=np.float32) / 16,
        "bq": rng.standard_normal(C).astype(np.float32) * 0.01,
        "Wk": rng.standard_normal((C, C), dtype=np.float32) / 16,
        "bk": rng.standard_normal(C).astype(np.float32) * 0.01,
        "Wv": rng.standard_normal((C, C), dtype=np.float32) / 16,
        "bv": rng.standard_normal(C).astype(np.float32) * 0.01,
        "Wo": rng.standard_normal((C, C), dtype=np.float32) / 16,
        "bo": rng.standard_normal(C).astype(np.float32) * 0.01,
    }
    out = kernel(**ins)
    # numpy reference
    x = ins["x"].reshape(8, C, N)
    q = np.einsum("oc,bcn->bon", ins["Wq"], x) + ins["bq"][None, :, None]
    k = np.einsum("oc,bcn->bon", ins["Wk"], x) + ins["bk"][None, :, None]
    v = np.einsum("oc,bcn->bon", ins["Wv"], x) + ins["bv"][None, :, None]
    q = q.reshape(8, NH, HD, N); k = k.reshape(8, NH, HD, N); v = v.reshape(8, NH, HD, N)
    sc = np.einsum("bhdn,bhdm->bhnm", q, k) / math.sqrt(HD)
    w = np.exp(sc - sc.max(-1, keepdims=True))
    w /= w.sum(-1, keepdims=True)
    o = np.einsum("bhnm,bhdm->bhnd", w, v)
    o = o.transpose(0, 1, 3, 2).reshape(8, C, N)
    exp = np.einsum("oc,bcn->bon", ins["Wo"], o) + ins["bo"][None, :, None] + x
    exp = exp.reshape(8, C, 32, 32)
    rel = np.linalg.norm(out - exp) / np.linalg.norm(exp)
    print("out", out.shape, "rel_err", rel)


# revision 6
# speedup vs baseline: 1.0236x; 1.0236x over previous
"""Multi-head self-attention Trainium2 kernel (Bass/Tile), v7.

Problem: x:(8,256,32,32), 8 heads, head_dim=32, N=H*W=1024.
Sharding: data-parallel over batch B=8 -> one batch element per NeuronCore.

Design vs v6 (135.8us local):
  - 32-pitch head layout: Qh/Kh [128,1024] x 2 tiles, head 4t+j at
    partitions 32j.  Halves Q/K projection streaming (no zero columns).
  - q-bias via DVE tensor_scalar_add (bq folded, scaled); no u-row/ones-row.
    S = (Wq x * s + bq s)^T (Wk x): bk/bq constants drop under softmax.
  - Steady state is ACT-paced: per (pair, mc): 2 score-pair MMs (K=32,
    2-way row-group concurrent), 2 exps [128,1024], 1 AV pair (M=33,
    2-way col-group concurrent, N=1024 streams).  PE/slot ~= 1.0us cold
    < 1.11us exp, so ACT never waits on PE even at K=4/8.
  - Garbage-matmul warm-up at t=0 (scratch SBUF) warms HAM during input DMA
    (an exp-paced stream never warms it; steady state runs at 1.2GHz).
  - Projections: K0/Q0 half-granular on the ps rotation feed the first
    score; V and K1/Q1 injected into pair-0 slot shadows.
  - PSUM: ps 2x[128,1024] (score double-buffer, 4 banks) + psO 2x[98,512]
    (per-jn AV accum so next pair's AV never waits the drain; esum rows
    ride at partitions 32/96 via VH ones-cols) + aux [128,*] (proj/prn/po
    transients, 2 banks) = 8 banks.  MM outputs are capped at 512 f32
    cols (one-bank ISA rule); concurrent same-bank MM writes wedge the
    device.
  - Per-pair epilogue: DVE copies psO -> OST (partition-aligned, no
    remap), DMA-gathers esum rows to base-0 ESUM (recip_approx_fast
    miscomputes on base!=0 APs), recip+bf16 cast, per-pair broadcast MM
    (prn), O1_p = OST*prn, pair-split output projection (WOP_p has zero
    rows at esum/junk positions so junk*0 stays clean), PART[mo]
    accumulated on DVE.  Pair 3 runs this chain jn-pipelined, copies
    split ACT/DVE, with quarter-wise output DMA on separate queues.
"""

import math

import numpy as np
import ml_dtypes

import concourse.bass as bass
import concourse.mybir as mybir
import concourse.tile as tile
from concourse import bacc
from concourse.bass_utils import run_bass_kernel_spmd

F32 = mybir.dt.float32
BF16 = mybir.dt.bfloat16
EXP = mybir.ActivationFunctionType.Exp

NH = 8
HD = 32
C = 256
N = 1024
NCORES = 8

_NC = None
LAST_RESULTS = None
import os as _os
KEEPALIVE = int(_os.environ.get("BASS_KEEPALIVE", "0"))


def _emit(tc, io):
    nc = tc.nc
    import contextlib

    ctx = contextlib.ExitStack()
    with ctx:
        pers = ctx.enter_context(tc.tile_pool(name="pers", bufs=1))
        etp = ctx.enter_context(tc.tile_pool(name="etp", bufs=3))
        psp = ctx.enter_context(tc.tile_pool(name="psp", bufs=1, space="PSUM"))

        def ptile(name, shape, dtype=F32):
            return pers.tile(shape, dtype, tag=name, name=name)

        # ---------------- persistent tiles ----------------
        XB = [ptile(f"XB{i}", [128, N], BF16) for i in range(2)]
        WQ = [ptile(f"WQ{i}", [128, C], BF16) for i in range(2)]
        WK = [ptile(f"WK{i}", [128, C], BF16) for i in range(2)]
        WV = [ptile(f"WV{i}", [128, C], BF16) for i in range(2)]
        WOP = [ptile(f"WOP{p}", [97, C], BF16) for p in range(4)]
        BQ = ptile("BQ", [128, 2])
        OHB = ptile("OHB", [2, 97], BF16)
        XPB = [ptile(f"XPB{i}", [128, N]) for i in range(2)]
        Qh = [ptile(f"Qh{t}", [128, N], BF16) for t in range(2)]
        Kh = [ptile(f"Kh{t}", [128, N], BF16) for t in range(2)]
        VH = [ptile(f"VH{mc}", [128, NH * 33], BF16) for mc in range(NH)]
        OST = [ptile(f"OST{p}", [97, N]) for p in range(4)]
        ESUM = [ptile(f"ESUM{p}", [2, N]) for p in range(4)]
        RECB = [ptile(f"RECB{p}", [2, N], BF16) for p in range(4)]
        O1 = [ptile(f"O1{p}", [97, N], BF16) for p in range(4)]
        PART = [ptile(f"PART{mo}", [128, N]) for mo in range(2)]
        WO2 = ptile("WO2", [128, C], BF16)
        O1P0 = ptile("O1P0", [128, N], BF16)
        SCR = ptile("SCR", [128, 640], BF16)

        # ---------------- warm-up + input DMA ----------------
        # garbage matmuls on scratch SBUF warm the PE HAM clock-gate to
        # 8/8 while the first-wave DMAs land (nothing depends on them)
        nc.vector.memset(SCR[:], 0.25)
        # OST rows 33-63 are never written by the psO drains; zero them once
        # so the O1 multiply reads initialized data (prn rows are 0 there)
        for p in range(4):
            nc.vector.memset(OST[p][32:64, :], 0.0)
        wup = psp.tile([128, N], F32, tag="aux", bufs=1, name="wup")
        for i in range(12):
            nc.tensor.matmul(
                wup[:, 0:512] if i % 2 == 0 else wup[:, 512:1024],
                SCR[:, 0:128], SCR[:, 128:640],
                start=True, stop=True,
            )
        nc.vector.tensor_copy(SCR[0:1, 0:32], wup[0:1, 0:32])

        # first-wave loads, issue spread over engine sequencers
        nc.sync.dma_start(XB[0][:], io["xb"][0:128, :])
        nc.sync.dma_start(XB[1][:], io["xb"][128:256, :])
        for i in range(2):
            nc.scalar.dma_start(WK[i][:], io["wk"][i * 128 : (i + 1) * 128, :])
        for i in range(2):
            nc.gpsimd.dma_start(WQ[i][:], io["wq"][i * 128 : (i + 1) * 128, :])
        nc.gpsimd.dma_start(BQ[:], io["bqc"])
        for i in range(2):
            nc.scalar.dma_start(WV[i][:], io["wv"][i * 128 : (i + 1) * 128, :])
        nc.gpsimd.dma_start(OHB[:], io["ohb"])

        # warm the ACT exp table while weights land
        warm = ptile("warm", [1, 32])
        nc.scalar.activation(warm[:], WK[0][0:1, 0:32], EXP)

        def late_dma():
            nc.sync.dma_start(WO2[:], io["wo2"])
            for p in (2, 3):
                nc.sync.dma_start(WOP[p][:], io["wop"][97 * p : 97 * p + 97, :])
            for i in range(2):
                nc.sync.dma_start(XPB[i][:], io["xpb"][i * 128 : (i + 1) * 128, :])

        # ---------------- building blocks ----------------
        def qk_half(t, jn, w, dst, bias, tag="aux"):
            """Half (512 pixels) of one 128-row chunk of the Q/K projection.
            Bias added on the psum->SBUF copy for Q."""
            js = slice(jn * 512, (jn + 1) * 512)
            pp = psp.tile(
                [128, 512], F32, tag=tag, bufs=1 if tag == "aux" else 2,
                name=f"pp{dst is Qh}{t}_{jn}",
            )
            for kc in range(2):
                nc.tensor.matmul(
                    pp[:], w[kc][:, t * 128 : (t + 1) * 128], XB[kc][:, js],
                    start=(kc == 0), stop=(kc == 1),
                )
            if bias is not None:
                nc.vector.tensor_scalar_add(dst[t][:, js], pp[:], bias[:, t : t + 1])
            else:
                nc.vector.tensor_copy(dst[t][:, js], pp[:])

        def pv_pack(i):
            """V^T for m-chunks 2i, 2i+1 in one aux psum alloc."""
            pvp = psp.tile([128, 512], F32, tag="aux", bufs=1, name=f"pvp{i}")
            for k in range(2):
                mc = 2 * i + k
                for kc in range(2):
                    nc.tensor.matmul(
                        pvp[:, k * 256 : (k + 1) * 256],
                        XB[kc][:, mc * 128 : (mc + 1) * 128],
                        WV[kc][:],
                        start=(kc == 0), stop=(kc == 1),
                    )
            for k in range(2):
                mc = 2 * i + k
                vh3 = VH[mc].rearrange("p (h c) -> p h c", c=33)
                pv3 = pvp[:, k * 256 : (k + 1) * 256].rearrange(
                    "p (h d) -> p h d", d=32
                )
                nc.vector.memset(VH[mc][:], 1.0)
                nc.vector.tensor_copy(vh3[:, :, 0:32], pv3[:, :, :])

        def score_mms(p, mc, jn):
            t, half = p // 2, p % 2
            ps = psp.tile([128, N], F32, tag="ps", bufs=2, name=f"ps{p}_{mc}_{jn}")
            for hh in range(2):
                base = 64 * half + 32 * hh
                nc.tensor.matmul(
                    ps[:, hh * 512 : (hh + 1) * 512],
                    Kh[t][base : base + 32, mc * 128 : (mc + 1) * 128],
                    Qh[t][base : base + 32, jn * 512 : (jn + 1) * 512],
                    start=True, stop=True,
                    tile_position=(base, 0),
                    skip_group_check=True,
                )
            return ps

        def exp_op(p, mc, jn, ps, et):
            nc.scalar.activation(et[:], ps[:], EXP)

        def av_mms(p, mc, jn, et, psO):
            for hh in range(2):
                h = 2 * p + hh
                nc.tensor.matmul(
                    psO[jn][64 * hh : 64 * hh + 33, :],
                    VH[mc][:, 33 * h : 33 * h + 33],
                    et[:, hh * 512 : (hh + 1) * 512],
                    start=(mc == 0), stop=(mc == 7),
                    tile_position=(0, 64 * hh),
                    skip_group_check=True,
                )

        def drain_jn(p, jn, psO, split_engines=False):
            """psO[jn] -> OST columns (partition-aligned) + esum DMA gather.
            Tiny esum-row copies go first so the gather DMAs fire early;
            for the tail-critical last drain the copies split across ACT
            (idle by then) and DVE."""
            js = slice(jn * 512, (jn + 1) * 512)
            copyA = nc.scalar.copy if split_engines else nc.vector.tensor_copy
            copyA(OST[p][32:33, js], psO[jn][32:33, :])
            nc.vector.tensor_copy(OST[p][96:97, js], psO[jn][96:97, :])
            nc.sync.dma_start(ESUM[p][0:1, js], OST[p][32:33, js], single_packet=True)
            nc.scalar.dma_start(ESUM[p][1:2, js], OST[p][96:97, js], single_packet=True)
            copyA(OST[p][0:32, js], psO[jn][0:32, :])
            nc.vector.tensor_copy(OST[p][64:96, js], psO[jn][64:96, :])

        def recip_jn(p, jn, cast_on_act=False):
            js = slice(jn * 512, (jn + 1) * 512)
            RECF = etp.tile([2, 512], F32, tag="recf", bufs=2, name=f"recf{p}{jn}")
            with nc.allow_low_precision("softmax denom recip (~1e-3 rel)"):
                nc.vector.reciprocal_approx_fast(RECF[:], ESUM[p][:, js])
            if cast_on_act:
                nc.scalar.copy(RECB[p][:, js], RECF[:])
            else:
                nc.vector.tensor_copy(RECB[p][:, js], RECF[:])

        def recip_pair(p):
            for jn in range(2):
                recip_jn(p, jn)

        prns = {}

        def prn_mms(p, jn=None, tag="aux"):
            """prn_p = OHB^T @ RECB[p] broadcast into 32-row blocks."""
            if p not in prns:
                prns[p] = psp.tile(
                    [128, N], F32, tag=tag, bufs=2 if tag == "ps" else 1,
                    name=f"prn{p}",
                )
            prn = prns[p]
            for j in ((0, 1) if jn is None else (jn,)):
                js = slice(j * 512, (j + 1) * 512)
                nc.tensor.matmul(
                    prn[0:97, js], OHB[:], RECB[p][:, js],
                    start=True, stop=True,
                    skip_group_check=True,
                )

        def norm_mul(p, jn=None):
            prn = prns[p]
            if jn is None:
                nc.vector.tensor_mul(O1[p][:], OST[p][:], prn[0:97, :])
            else:
                js = slice(jn * 512, (jn + 1) * 512)
                nc.vector.tensor_mul(O1[p][:, js], OST[p][:, js], prn[0:97, js])

        OUT_QUEUES = [nc.sync, nc.scalar, nc.gpsimd, nc.sync]

        po3 = {}

        def po_unit(p, mo, tag="aux", jn=None):
            """Pair p's contribution to output chunk mo; PART accumulates
            on DVE (PART[mo] starts as XPB[mo] + pair0).  For the final
            pair (jn-split) the adds/DMAs go out quarter-wise on separate
            queues."""
            if p == 3:
                if mo not in po3:
                    po3[mo] = psp.tile(
                        [128, N], F32, tag=tag, bufs=2, name=f"po3_{mo}"
                    )
                po = po3[mo]
                js = slice(jn * 512, (jn + 1) * 512)
                nc.tensor.matmul(
                    po[:, js], WOP[p][:, mo * 128 : (mo + 1) * 128],
                    O1[p][:, js],
                    start=True, stop=True,
                )
                nc.vector.tensor_add(PART[mo][:, js], po[:, js], PART[mo][:, js])
                OUT_QUEUES[2 * mo + jn].dma_start(
                    io["out"][mo * 128 : (mo + 1) * 128, js],
                    PART[mo][:, js],
                )
                return
            raise AssertionError("mid-stream pairs use po_half")

        pos = {}

        def po_half(p, mo, jn):
            """One jn-half of pair p's output-projection contribution."""
            if (p, mo) not in pos:
                pos[(p, mo)] = psp.tile(
                    [128, N], F32, tag="aux", bufs=1, name=f"po{p}_{mo}"
                )
            po = pos[(p, mo)]
            js = slice(jn * 512, (jn + 1) * 512)
            nc.tensor.matmul(
                po[:, js], WOP[p][:, mo * 128 : (mo + 1) * 128],
                O1[p][:, js],
                start=True, stop=True,
            )
            nc.vector.tensor_add(PART[mo][:, js], po[:, js], PART[mo][:, js])

        def remap_pair(p):
            """Pack pair p's normalized O rows into the kc0 tile (heads
            2p, 2p+1 -> rows 64p..64p+63) via DMA partition remap."""
            nc.gpsimd.dma_start(O1P0[64 * p : 64 * p + 32, :], O1[p][0:32, :])
            nc.gpsimd.dma_start(O1P0[64 * p + 32 : 64 * p + 64, :], O1[p][64:96, :])

        def po_kc0_half(mo, jn):
            """Heads 0-3 (pairs 0+1) output-projection contribution as one
            packed K=128 MM; initializes PART with XPB."""
            if ("kc0", mo) not in pos:
                pos[("kc0", mo)] = psp.tile(
                    [128, N], F32, tag="aux", bufs=1, name=f"pokc0_{mo}"
                )
            po = pos[("kc0", mo)]
            js = slice(jn * 512, (jn + 1) * 512)
            nc.tensor.matmul(
                po[:, js], WO2[:, mo * 128 : (mo + 1) * 128], O1P0[:, js],
                start=True, stop=True,
            )
            nc.vector.tensor_add(PART[mo][:, js], po[:, js], XPB[mo][:, js])

        # ---------------- emission schedule ----------------
        # first-score prerequisites, half-granular on the ps rotation
        qk_half(0, 0, WK, Kh, None, tag="ps")
        qk_half(0, 0, WQ, Qh, BQ, tag="ps")

        # deferred PE work injected into slot shadows
        deferred = {
            (0, 0): [lambda: pv_pack(0)],
            (0, 1): [lambda: pv_pack(1)],
            (0, 2): [lambda: pv_pack(2)],
            (0, 3): [lambda: pv_pack(3)],
            (0, 4): [lambda: qk_half(1, 0, WK, Kh, None)],
            (0, 5): [late_dma, lambda: qk_half(1, 0, WQ, Qh, BQ)],
            (0, 6): [lambda: qk_half(1, 1, WK, Kh, None)],
            (0, 7): [lambda: qk_half(1, 1, WQ, Qh, BQ)],
        }
        for q in range(3):
            deferred[(q + 1, 1)] = [lambda q=q: prn_mms(q, jn=0)]
            deferred[(q + 1, 2)] = [lambda q=q: (prn_mms(q, jn=1),
                                                 norm_mul(q))]
        deferred[(1, 3)] = [lambda: remap_pair(0)]
        deferred[(2, 3)] = [lambda: remap_pair(1)]
        deferred[(2, 4)] = [lambda: po_kc0_half(0, 0)]
        deferred[(2, 5)] = [lambda: po_kc0_half(0, 1)]
        deferred[(2, 6)] = [lambda: po_kc0_half(1, 0)]
        deferred[(3, 0)] = [lambda: po_kc0_half(1, 1)]
        deferred[(3, 3)] = [lambda: po_half(2, 0, 0)]
        deferred[(3, 4)] = [lambda: po_half(2, 0, 1)]
        deferred[(3, 5)] = [lambda: po_half(2, 1, 0)]
        deferred[(3, 6)] = [lambda: po_half(2, 1, 1)]

        tail3 = []
        for p in range(4):
            psO = [
                psp.tile([98, 512], F32, tag="psO", bufs=2, name=f"psO{p}_{jn}")
                for jn in range(2)
            ]
            for mc in range(8):
                ets = []
                for jn in range(2):
                    if (p, mc, jn) == (0, 0, 1):
                        # rest of chunk-0 projections right behind score 0
                        qk_half(0, 1, WK, Kh, None, tag="ps")
                        qk_half(0, 1, WQ, Qh, BQ, tag="ps")
                    ps = score_mms(p, mc, jn)
                    et = etp.tile([128, N], BF16, tag="et", name=f"et{p}_{mc}_{jn}")
                    exp_op(p, mc, jn, ps, et)
                    ets.append(et)
                for fn in deferred.get((p, mc), ()):
                    fn()
                for jn in range(2):
                    av_mms(p, mc, jn, ets[jn], psO)
                    if mc == 7 and p < 3:
                        drain_jn(p, jn, psO)
                    elif mc == 7:
                        # jn-pipelined tail, DVE/ACT balanced: esum copies +
                        # gather DMAs fire first; jn1's early copies run on
                        # the just-idle ACT; the recip DMA-wait is filled
                        # with block copies; casts go to ACT so DVE keeps
                        # only recip/mul/add on the critical chain.
                        js = slice(jn * 512, (jn + 1) * 512)
                        copyA = nc.scalar.copy if jn else nc.vector.tensor_copy
                        copyA(OST[3][32:33, js], psO[jn][32:33, :])
                        nc.vector.tensor_copy(OST[3][96:97, js], psO[jn][96:97, :])
                        nc.sync.dma_start(ESUM[3][0:1, js], OST[3][32:33, js], single_packet=True)
                        nc.scalar.dma_start(ESUM[3][1:2, js], OST[3][96:97, js], single_packet=True)
                        copyA(OST[3][0:32, js], psO[jn][0:32, :])
                        tail3.append(jn)

                def run_tail3():
                    # recips first (their DMA inputs land earliest), then
                    # blockB copies + both prn MMs ahead of any po unit so
                    # neither jn's normalize waits on the other's project
                    for jn in tail3:
                        recip_jn(3, jn, cast_on_act=True)
                    for jn in tail3:
                        js = slice(jn * 512, (jn + 1) * 512)
                        nc.vector.tensor_copy(OST[3][64:96, js], psO[jn][64:96, :])
                        prn_mms(3, jn=jn, tag="aux")
                    for jn in tail3:
                        norm_mul(3, jn)
                        po_unit(3, 0, tag="ps", jn=jn)
                        po_unit(3, 1, tag="ps", jn=jn)
                    tail3.clear()

                if p == 3 and mc == 7:
                    run_tail3()
                for _ in range(KEEPALIVE):
                    nc.tensor.ldweights(SCR[:, 0:128])
            if p < 3:
                recip_pair(p)


def build_nc():
    nc = bacc.Bacc("TRN2", target_bir_lowering=False, debug=False)
    io = {}
    for name, shape, dt_ in [
        ("xb", (C, N), BF16),
        ("wq", (C, C), BF16),
        ("wk", (C, C), BF16),
        ("wv", (C, C), BF16),
        ("wop", (388, C), BF16),
        ("wo2", (128, C), BF16),
        ("bqc", (128, 2), F32),
        ("ohb", (2, 97), BF16),
        ("xpb", (C, N), F32),
    ]:
        io[name] = nc.dram_tensor(name, shape, dt_, kind="ExternalInput").ap()
    io["out"] = nc.dram_tensor("out", (C, N), F32, kind="ExternalOutput").ap()
    with tile.TileContext(nc) as tc:
        _emit(tc, io)
    nc.finalize()
    return nc


def host_prep(x, Wq, bq, Wk, bk, Wv, bv, Wo, bo):
    """Build per-core input maps (numpy only)."""
    bf16 = ml_dtypes.bfloat16
    x = np.ascontiguousarray(np.asarray(x, np.float32))
    Wq, bq = np.asarray(Wq, np.float32), np.asarray(bq, np.float32)
    Wk = np.asarray(Wk, np.float32)
    Wv, bv = np.asarray(Wv, np.float32), np.asarray(bv, np.float32)
    Wo, bo = np.asarray(Wo, np.float32), np.asarray(bo, np.float32)
    s = 1.0 / math.sqrt(HD)

    wq_hat = np.ascontiguousarray(Wq.T * s)          # [C, C] col 32h+d
    wk_hat = np.ascontiguousarray(Wk.T)
    bqc = (bq * s).reshape(2, 128).T                  # [128, 2] chunk cols
    wv_hat = np.ascontiguousarray(Wv.T)

    woT = Wo.T                                        # [C(d), C(out)]
    wop = np.zeros((388, C), np.float32)
    for p in range(4):
        wop[97 * p + 0 : 97 * p + 32, :] = woT[32 * (2 * p) : 32 * (2 * p) + 32, :]
        wop[97 * p + 64 : 97 * p + 96, :] = woT[32 * (2 * p + 1) : 32 * (2 * p + 1) + 32, :]

    wo2 = np.ascontiguousarray(woT[0:128, :])

    ohb = np.zeros((2, 97), np.float32)
    ohb[0, 0:32] = 1.0
    ohb[1, 64:96] = 1.0

    bo2 = Wo @ bv + bo

    common = {
        "wq": wq_hat.astype(bf16),
        "wk": wk_hat.astype(bf16),
        "wv": wv_hat.astype(bf16),
        "wop": wop.astype(bf16),
        "wo2": wo2.astype(bf16),
        "bqc": np.ascontiguousarray(bqc),
        "ohb": ohb.astype(bf16),
    }

    B = x.shape[0]
    in_maps = []
    for b in range(B):
        xb = np.ascontiguousarray(x[b].reshape(C, N))
        m = dict(common)
        m["xb"] = xb.astype(bf16)
        m["xpb"] = np.ascontiguousarray(xb + bo2[:, None])
        in_maps.append(m)
    return in_maps


def kernel(x, Wq, bq, Wk, bk, Wv, bv, Wo, bo):
    global _NC, LAST_RESULTS
    if _NC is None:
        _NC = build_nc()
    in_maps = host_prep(x, Wq, bq, Wk, bk, Wv, bv, Wo, bo)
    res = run_bass_kernel_spmd(_NC, in_maps, core_ids=list(range(NCORES)))
    LAST_RESULTS = res
    out = np.stack([r["out"] for r in res.results], axis=0)
    return out.reshape(NCORES, C, 32, 32).astype(np.float32)


if __name__ == "__main__":
    rng = np.random.default_rng(0)
    ins = {
        "x": rng.standard_normal((8, C, 32, 32), dtype=np.float32),
        "Wq": rng.standard_normal((C, C), dtype=np.float32) / 16,
        "bq": rng.standard_normal(C).astype(np.float32) * 0.01,
        "Wk": rng.standard_normal((C, C), dtype=np.float32) / 16,
        "bk": rng.standard_normal(C).astype(np.float32) * 0.01,
        "Wv": rng.standard_normal((C, C), dtype=np.float32) / 16,
        "bv": rng.standard_normal(C).astype(np.float32) * 0.01,
        "Wo": rng.standard_normal((C, C), dtype=np.float32) / 16,
        "bo": rng.standard_normal(C).astype(np.float32) * 0.01,
    }
    out = kernel(**ins)
    # numpy reference
    x = ins["x"].reshape(8, C, N)
    q = np.einsum("oc,bcn->bon", ins["Wq"], x) + ins["bq"][None, :, None]
    k = np.einsum("oc,bcn->bon", ins["Wk"], x) + ins["bk"][None, :, None]
    v = np.einsum("oc,bcn->bon", ins["Wv"], x) + ins["bv"][None, :, None]
    q = q.reshape(8, NH, HD, N); k = k.reshape(8, NH, HD, N); v = v.reshape(8, NH, HD, N)
    sc = np.einsum("bhdn,bhdm->bhnm", q, k) / math.sqrt(HD)
    w = np.exp(sc - sc.max(-1, keepdims=True))
    w /= w.sum(-1, keepdims=True)
    o = np.einsum("bhnm,bhdm->bhnd", w, v)
    o = o.transpose(0, 1, 3, 2).reshape(8, C, N)
    exp = np.einsum("oc,bcn->bon", ins["Wo"], o) + ins["bo"][None, :, None] + x
    exp = exp.reshape(8, C, 32, 32)
    rel = np.linalg.norm(out - exp) / np.linalg.norm(exp)
    print("out", out.shape, "rel_err", rel)
